# revision 1
# baseline (speedup 1.0000x reference)
"""Bass/Tile kernel for nn_CloudCrop (cylinder-query + gather + SharedMLP + max-pool).

Sharding: 8 cores = 4 batches x 2 query-halves. Each core handles 512 queries
against its batch's full 1024-point cloud.

Per-core pipeline (4 query-tiles of 128):
  1. PE: rotate all points into each query's frame (fp32 matmuls, K=3).
  2. ACT: scaled squares; GPSIMD: combine -> g = max((y^2+z^2)/R^2, x'^2/h^2).
  3. ACT: sign(1-g) (+count via accum); DVE: score = desc * sign;
     4x (max8 + match_replace) extracts the first 32 in-cylinder point indices.
  4. idx staging roundtrip through DRAM into the 16-partition-wrapped layout.
  5. SWDGE dma_gather (SBUF->SBUF, transpose): pulls Z-rows (256ch bf16 +
     xyz) for all 4096 samples, channel-major.
  6. PE: per-query A = R @ w1x^T/RAD (K=3); block-diag xyz rhs built via DRAM
     diagonal-stride staging; y1 = relu(Zg + A.x + b1) with identity-inject.
  7. PE: layer 2 (bf16); DVE: max over 32 samples; ACT: relu + bias; DMA out.
"""
import sys

_RL = "/opt/trn_rl_repo"
if _RL not in sys.path:
    sys.path.insert(0, _RL)

from contextlib import ExitStack

import numpy as np
import ml_dtypes

import concourse.bass as bass
import concourse.bacc as bacc
import concourse.mybir as mybir
import concourse.tile as tile
from concourse import library_config

F32 = mybir.dt.float32
BF16 = mybir.dt.bfloat16
F16 = mybir.dt.float16
I16 = mybir.dt.int16
U8 = mybir.dt.uint8
AL = mybir.AluOpType
AF = mybir.ActivationFunctionType
AX = mybir.AxisListType

RADIUS = 0.05
HMIN = -0.02
HMAX = 0.04
NS = 32
CIN = 512
CMID = 256
COUT = 256
EPS = 1e-5
B = 4
NQ = 1024
M = 1024
NT = 4          # query tiles per core
QPC = 512       # queries per core
G8 = 8          # groups per query tile
QG = 16         # queries per group
CX = (HMIN + HMAX) / 2.0
HH = (HMAX - HMIN) / 2.0
NCORES = 8

bf16 = ml_dtypes.bfloat16


def build_nc(_stage=4, reps=1):
    nc = bacc.Bacc("TRN2", target_bir_lowering=False, num_devices=NCORES)
    I = {}

    def din(name, shape, dt):
        I[name] = nc.dram_tensor(name, shape, dt, kind="ExternalInput").ap()

    din("xyzT", [3, M], F32)
    din("xyzP", [128, 8, 3], F32)
    din("feat", [CIN, M], F32)
    din("w1fT", [128, 4, CMID], F32)
    din("w1xA", [3, CMID], F32)
    din("RrowT", [3, NT, 3, 128], F32)
    din("RcolT", [3, NT, G8, 6 * QG], F32)
    din("Rq9", [128, NT, 9], F32)
    din("cQ9", [128, NT, 9], F32)
    din("cneg32", [3, QPC, NS], BF16)
    din("w2T", [128, 2, 2, 128], BF16)
    din("b1", [128, 2], F32)
    din("b2", [128, 2], F32)
    din("desc16", [128, M], F16)
    din("siota", [128, NS], F32)
    din("qglob", [128, NT, NS], I16)
    din("ident", [128, 128], BF16)
    OUT = nc.dram_tensor("out", [COUT, QPC], F32, kind="ExternalOutput").ap()

    with ExitStack() as ctx:
        tc = ctx.enter_context(tile.TileContext(nc))
        res = ctx.enter_context(tc.tile_pool(name="res", bufs=1))
        wk = ctx.enter_context(tc.tile_pool(name="wk", bufs=2))
        wk1 = ctx.enter_context(tc.tile_pool(name="wk1", bufs=1))
        drm = ctx.enter_context(tc.tile_pool(name="drm", bufs=2, space="DRAM"))
        drs = ctx.enter_context(tc.tile_pool(name="drs", bufs=1, space="DRAM"))
        p_rot = ctx.enter_context(tc.tile_pool(name="prot", bufs=1, space="PSUM"))
        p_y1 = ctx.enter_context(tc.tile_pool(name="py1", bufs=1, space="PSUM"))
        p_y2 = ctx.enter_context(tc.tile_pool(name="py2", bufs=2, space="PSUM"))
        p_sm = ctx.enter_context(tc.tile_pool(name="psml", bufs=1, space="PSUM"))

        nc.gpsimd.load_library(library_config.mlp)

        # ---------- residents ----------
        def rload(name, shape, dt, src=None):
            t = res.tile(shape, dt, tag="res_" + name)
            nc.sync.dma_start(out=t[:], in_=src if src is not None else I[name])
            return t

        xyzT_s = rload("xyzT", [3, M], F32)
        w1fT_s = rload("w1fT", [128, 4, CMID], F32)
        w1xA_s = rload("w1xA", [3, CMID], F32)
        RrowT_s = rload("RrowT", [3, NT, 3, 128], F32)
        RcolT_s = rload("RcolT", [3, NT, G8, 6 * QG], F32)
        Rq9_s = rload("Rq9", [128, NT, 9], F32)
        cQ9_s = rload("cQ9", [128, NT, 9], F32)
        w2_s = rload("w2T", [128, 2, 2, 128], BF16)
        b1_s = rload("b1", [128, 2], F32)
        b2_s = rload("b2", [128, 2], F32)
        desc_s = rload("desc16", [128, M], F16)
        siota_s = rload("siota", [128, NS], F32)
        qglob_s = rload("qglob", [128, NT, NS], I16)
        ident_s = rload("ident", [128, 128], BF16)

        feat_s = res.tile([128, 4, M], F32, tag='res_feat')
        for kc in range(4):
            nc.sync.dma_start(out=feat_s[:, kc], in_=I["feat"][kc * 128:(kc + 1) * 128, :])

        # ---------- Z table (point-major, bf16): [m%128, m//128, 384] ----------
        repctx = tc.For_i(0, reps, 1) if reps > 1 else None
        if repctx is not None:
            repctx.__enter__()
        ZT = res.tile([128, 8, 384], BF16, tag='res_ZT')
        nc.scalar.memzero(ZT[:])
        for mt in range(8):
            pz = p_sm.tile([128, CMID], F32, tag="sm")
            for kc in range(4):
                nc.tensor.matmul(out=pz[:], lhsT=feat_s[:, kc, mt * 128:(mt + 1) * 128],
                                 rhs=w1fT_s[:, kc], start=(kc == 0), stop=(kc == 3))
            nc.scalar.copy(out=ZT[:, mt, 0:CMID], in_=pz[:])
        nc.gpsimd.dma_start(out=ZT[:, :, CMID:CMID + 3], in_=I["xyzP"])

        # ---------- block-diag DRAM staging (zeros written once) ----------
        zsrc = res.tile([96, G8 * 512], BF16, tag='res_zsrc')
        nc.vector.memset(zsrc[:], 0.0)
        rbdD = []
        for i in range(2):
            d = drs.tile([96, G8, 512], BF16, tag="rbdD%d" % i)
            nc.sync.dma_start(out=d[:], in_=zsrc[:])
            rbdD.append(d)

        # ---------- per query tile ----------
        for t in range(NT):
            # Rc[k] = sum_j R[k,j] * c[j]
            rc9 = wk.tile([128, 3, 3], F32, tag="rc9")
            nc.vector.tensor_tensor(
                out=rc9[:],
                in0=Rq9_s[:, t].rearrange("p (a b) -> p a b", a=3),
                in1=cQ9_s[:, t].rearrange("p (a b) -> p a b", a=3),
                op=AL.mult)
            rcb = wk.tile([128, 8], F32, tag="rcb")
            nc.vector.tensor_reduce(out=rcb[:, 0:3], in_=rc9[:], axis=AX.X, op=AL.add)
            nc.vector.tensor_scalar(out=rcb[:, 3:4], in0=rcb[:, 0:1],
                                    scalar1=-1.0 / HH, scalar2=-CX / HH,
                                    op0=AL.mult, op1=AL.add)
            nc.vector.tensor_scalar(out=rcb[:, 4:6], in0=rcb[:, 1:3],
                                    scalar1=-1.0 / RADIUS, scalar2=None, op0=AL.mult)

            # rotations + scaled squares
            sq = wk1.tile([128, 3, M], F32, tag="sq")
            for r in range(3):
                pr = p_rot.tile([128, M], F32, tag="rot")
                for h in range(2):
                    nc.tensor.matmul(out=pr[:, h * 512:(h + 1) * 512],
                                     lhsT=RrowT_s[:, t, r],
                                     rhs=xyzT_s[:, h * 512:(h + 1) * 512],
                                     start=True, stop=True)
                scale = (1.0 / HH) if r == 0 else (1.0 / RADIUS)
                nc.scalar.activation(out=sq[:, r], in_=pr[:], func=AF.Square,
                                     bias=rcb[:, 3 + r:4 + r], scale=scale)

            # g = max(y2+z2, x2); sign; score; count
            tgg = wk1.tile([128, 2, M], F32, tag="tg")
            nc.vector.tensor_tensor(out=tgg[:, 0], in0=sq[:, 1], in1=sq[:, 2], op=AL.add)
            nc.vector.tensor_tensor(out=tgg[:, 1], in0=tgg[:, 0], in1=sq[:, 0], op=AL.max)
            sgn = wk.tile([128, M], F16, tag="sgn")
            cnt_acc = wk.tile([128, 1], F32, tag="cnta")
            nc.scalar.activation(out=sgn[:], in_=tgg[:, 1], func=AF.Sign,
                                 bias=1.0, scale=-1.0, accum_out=cnt_acc[:])
            score = wk.tile([128, M], F16, tag="score")
            nc.vector.tensor_tensor(out=score[:], in0=desc_s[:], in1=sgn[:], op=AL.mult)

            # extract first-32 indices
            v32 = wk.tile([128, NS], F16, tag="v32")
            for r4 in range(4):
                nc.vector.max(out=v32[:, r4 * 8:(r4 + 1) * 8], in_=score[:])
                if r4 < 3:
                    nc.vector.match_replace(out=score[:],
                                            in_to_replace=v32[:, r4 * 8:(r4 + 1) * 8],
                                            in_values=score[:], imm_value=-3000.0)
            nc.vector.tensor_scalar(out=rcb[:, 6:7], in0=cnt_acc[:],
                                    scalar1=0.5, scalar2=512.0, op0=AL.mult, op1=AL.add)
            cond = wk.tile([128, NS], U8, tag="cond")
            nc.vector.tensor_scalar(out=cond[:], in0=siota_s[:], scalar1=rcb[:, 6:7],
                                    scalar2=None, op0=AL.is_lt)
            mvf = wk.tile([128, NS], F32, tag="mvf")
            nc.vector.tensor_scalar(out=mvf[:], in0=v32[:], scalar1=-1.0, scalar2=2048.0,
                                    op0=AL.mult, op1=AL.add)
            mvi = wk.tile([128, NS], I16, tag="mvi")
            nc.vector.tensor_scalar(out=mvi[:], in0=mvf[:], scalar1=1023.0, scalar2=None,
                                    op0=AL.min)
            idx16 = wk.tile([128, NS], I16, tag="idx16")
            nc.vector.tensor_copy(out=idx16[:], in_=qglob_s[:, t])
            nc.vector.copy_predicated(out=idx16[:], mask=cond[:], data=mvi[:])

            if _stage <= 1:
                dbg = wk.tile([128, NS], F32, tag="dbg")
                nc.vector.tensor_copy(out=dbg[:], in_=idx16[:])
                nc.sync.dma_start(out=OUT[0:128, t * NS:(t + 1) * NS], in_=dbg[:])
                continue
            # stage roundtrip -> wrapped idx layout, replicated to 8 core-blocks
            stage = drm.tile([128, NS], I16, tag="stage")
            nc.sync.dma_start(out=stage[:], in_=idx16[:])
            wrap = stage[:].rearrange("q (h p) -> p (q h)", h=2, p=16)
            idxw = wk.tile([128, 256], I16, tag="idxw")
            for blk in range(8):
                nc.sync.dma_start(out=idxw[blk * 16:(blk + 1) * 16, :], in_=wrap)

            # gather: channel-major [c%128, c//128, sample]; row 2 parts 0-2 = xyz
            if _stage == 15:
                dbg15 = wk.tile([128, 256], F32, tag="dbg15")
                nc.vector.tensor_copy(out=dbg15[:], in_=idxw[:])
                nc.sync.dma_start(out=OUT[0:128, t * 128:(t + 1) * 128], in_=dbg15[:, 0:128])
                continue
            gz = wk1.tile([128, 3, NT * M], BF16, tag="gz")
            nc.gpsimd.dma_gather(out_ap=gz[:], in_ap=ZT[:], idxs_ap=idxw[:],
                                 num_idxs=4096, num_idxs_reg=4096, elem_size=384,
                                 transpose=True, sbuf_tokens_per_rank=128,
                                 sbuf_free_dim_per_rank=768, single_packet=False)

            if _stage <= 2:
                dbg2 = wk.tile([128, 128], F32, tag="dbg2")
                nc.vector.tensor_copy(out=dbg2[:], in_=gz[:, 0, 0:128])
                nc.sync.dma_start(out=OUT[0:128, t * 128:(t + 1) * 128], in_=dbg2[:])
                continue
            # A matrices, [row=(6q'+jj), o]
            Ab = wk.tile([96, G8, CMID], BF16, tag="Ab")
            for g in range(G8):
                pa = p_sm.tile([128, CMID], F32, tag="sm")
                nc.tensor.matmul(out=pa[0:96, :], lhsT=RcolT_s[:, t, g], rhs=w1xA_s[:],
                                 start=True, stop=True)
                nc.scalar.copy(out=Ab[:, g], in_=pa[0:96, :])

            # block-diagonal rhs via DRAM diagonal strides
            rd = rbdD[t % 2]
            rflat = rd[:].rearrange("r g c -> (r g c)")
            xyzD = drm.tile([3, NT * M], BF16, tag="xyzD")
            nc.sync.dma_start(out=xyzD[:], in_=gz[0:3, 2, :])
            for jj in range(3):
                srcx = xyzD[jj, :].rearrange("(g q s) -> q g s", g=G8, q=QG, s=NS)
                dstx = bass.AP(tensor=rflat.tensor, offset=rflat.offset + jj * 4096,
                               ap=[[6 * 4096 + 32, 16], [512, 8], [1, 32]])
                nc.sync.dma_start(out=dstx, in_=srcx)
                srcc = I["cneg32"][jj, t * 128:(t + 1) * 128, :].rearrange(
                    "(g q) s -> q g s", g=G8, q=QG)
                dstc = bass.AP(tensor=rflat.tensor, offset=rflat.offset + (3 + jj) * 4096,
                               ap=[[6 * 4096 + 32, 16], [512, 8], [1, 32]])
                nc.sync.dma_start(out=dstc, in_=srcc)
            rbd = wk.tile([96, G8, 512], BF16, tag="rbd")
            nc.sync.dma_start(out=rbd[:], in_=rd[:])

            # y1 = relu(Zg + A.x + b1)
            y1 = wk1.tile([128, 2, NT * M], BF16, tag="y1")
            for oc in range(2):
                for g in range(G8):
                    py1 = p_y1.tile([128, 512], F32, tag="y1p")
                    nc.tensor.matmul(out=py1[:], lhsT=ident_s[:],
                                     rhs=gz[:, oc, g * 512:(g + 1) * 512],
                                     start=True, stop=False)
                    nc.tensor.matmul(out=py1[:], lhsT=Ab[:, g, oc * 128:(oc + 1) * 128],
                                     rhs=rbd[:, g], start=False, stop=True)
                    nc.scalar.activation(out=y1[:, oc, g * 512:(g + 1) * 512], in_=py1[:],
                                         func=AF.Relu, bias=b1_s[:, oc:oc + 1], scale=1.0)

            if _stage <= 3:
                dbg3 = wk.tile([128, 128], F32, tag="dbg3")
                nc.vector.tensor_copy(out=dbg3[:], in_=y1[:, 0, 0:128])
                nc.sync.dma_start(out=OUT[0:128, t * 128:(t + 1) * 128], in_=dbg3[:])
                continue
            # layer 2 + max over 32 samples + relu+bias
            mx = wk.tile([128, 2, 128], F32, tag="mx")
            for oc2 in range(2):
                for gp in range(G8 // 2):
                    py2 = p_y2.tile([128, 1024], F32, tag="y2p")
                    for kc in range(2):
                        for ns in range(2):
                            nc.tensor.matmul(
                                out=py2[:, ns * 512:(ns + 1) * 512],
                                lhsT=w2_s[:, kc, oc2],
                                rhs=y1[:, kc, (2 * gp + ns) * 512:(2 * gp + ns + 1) * 512],
                                start=(kc == 0), stop=(kc == 1))
                    nc.vector.tensor_reduce(
                        out=mx[:, oc2, gp * 2 * QG:(gp + 1) * 2 * QG],
                        in_=py2[:].rearrange("p (q s) -> p q s", s=NS),
                        axis=AX.X, op=AL.max)
            outv = wk.tile([128, 2, 128], F32, tag="outv")
            for oc2 in range(2):
                nc.scalar.activation(out=outv[:, oc2], in_=mx[:, oc2], func=AF.Relu,
                                     bias=b2_s[:, oc2:oc2 + 1], scale=1.0)
                nc.sync.dma_start(out=OUT[oc2 * 128:(oc2 + 1) * 128, t * 128:(t + 1) * 128],
                                  in_=outv[:, oc2])
        if repctx is not None:
            repctx.__exit__(None, None, None)
    return nc


def prep_inputs(inputs):
    xyz = np.asarray(inputs["seed_xyz"], dtype=np.float32)
    feat = np.asarray(inputs["seed_features"], dtype=np.float32)
    rot = np.asarray(inputs["vp_rot"], dtype=np.float32)

    def fold(g, b, m, v):
        s = (np.asarray(g, np.float32) /
             np.sqrt(np.asarray(v, np.float32) + np.float32(EPS))).astype(np.float32)
        return s, (np.asarray(b, np.float32) - np.asarray(m, np.float32) * s).astype(np.float32)

    s1, bb1 = fold(inputs["g1"], inputs["b1"], inputs["m1"], inputs["v1"])
    s2, bb2 = fold(inputs["g2"], inputs["b2"], inputs["m2"], inputs["v2"])
    w1s = (np.asarray(inputs["w1"], np.float32) * s1[:, None]).astype(np.float32)
    w1x, w1f = w1s[:, :3], w1s[:, 3:]
    w2s = (np.asarray(inputs["w2"], np.float32) * s2[:, None]).astype(np.float32)

    desc16 = np.broadcast_to(
        (2048.0 - np.arange(M, dtype=np.float32)).astype(np.float16), (128, M)).copy()
    siota = np.broadcast_to(np.arange(NS, dtype=np.float32), (128, NS)).copy()
    ident = np.eye(128, dtype=np.float32).astype(bf16)
    w2T = np.ascontiguousarray(
        w2s.T.reshape(2, 128, 2, 128).transpose(1, 0, 2, 3)).astype(bf16)
    w1fT = np.ascontiguousarray(w1f.T.reshape(4, 128, CMID).transpose(1, 0, 2))
    w1xA = np.ascontiguousarray(w1x.T / np.float32(RADIUS))
    b1h = np.ascontiguousarray(bb1.reshape(2, 128).T)
    b2h = np.ascontiguousarray(bb2.reshape(2, 128).T)

    ins = []
    for c in range(NCORES):
        b, half = c // 2, c % 2
        X = xyz[b]
        Rt = rot[b]
        qs = slice(half * QPC, (half + 1) * QPC)
        Rq = Rt[qs]          # (512, 3, 3)  R[j, k] (einsum 'bnsj,bnjk')
        cq = X[qs]           # (512, 3)
        d = {}
        d["xyzT"] = np.ascontiguousarray(X.T)
        d["xyzP"] = np.ascontiguousarray(X.reshape(8, 128, 3).transpose(1, 0, 2))
        d["feat"] = feat[b]
        d["w1fT"] = w1fT
        d["w1xA"] = w1xA
        # cylinder-query rotation: x_k = sum_j rot[k, j] rel_j
        Rr = Rq.reshape(NT, 128, 3, 3)   # [t, q, k(row), j(col)]
        d["RrowT"] = np.ascontiguousarray(Rr.transpose(3, 0, 2, 1))  # [j, t, k, q]
        # layer-1 rotation A[j, o] = sum_k rot[j, k] w1x[o, k] / RAD
        Rg = Rq.reshape(NT, G8, QG, 3, 3)  # [t, g, q', j, k]
        rcol = np.zeros((3, NT, G8, 6 * QG), np.float32)
        for jj in range(6):
            rcol[:, :, :, jj::6] = Rg[:, :, :, jj % 3, :].transpose(3, 0, 1, 2)
        d["RcolT"] = rcol
        d["Rq9"] = np.ascontiguousarray(Rq.reshape(NT, 128, 9).transpose(1, 0, 2))
        d["cQ9"] = np.ascontiguousarray(
            np.tile(cq.reshape(NT, 128, 3), (1, 1, 3)).transpose(1, 0, 2))
        d["cneg32"] = np.ascontiguousarray(
            np.broadcast_to((-cq.T)[:, :, None], (3, QPC, NS))).astype(bf16)
        d["w2T"] = w2T
        d["b1"] = b1h
        d["b2"] = b2h
        d["desc16"] = desc16
        d["siota"] = siota
        qg = (half * QPC + np.arange(QPC, dtype=np.int16)).reshape(NT, 128)
        d["qglob"] = np.ascontiguousarray(
            np.broadcast_to(qg.T[:, :, None], (128, NT, NS))).astype(np.int16)
        d["ident"] = ident
        ins.append(d)
    return ins


def assemble(results):
    out = np.zeros((B, COUT, NQ), np.float32)
    for c in range(NCORES):
        b, half = c // 2, c % 2
        out[b, :, half * QPC:(half + 1) * QPC] = results[c]["out"]
    return out


_COMPILED = None


def _get_compiled():
    global _COMPILED
    if _COMPILED is None:
        nc = build_nc()
        nc.compile()
        _COMPILED = nc
    return _COMPILED


def kernel(**inputs):
    """Full-input entry point: shards across 8 NeuronCores, returns (B, 256, N) f32."""
    from concourse.bass_utils import run_bass_kernel_spmd
    nc = _get_compiled()
    ins = prep_inputs(inputs)
    res = run_bass_kernel_spmd(nc, ins, core_ids=list(range(NCORES)))
    return assemble(res.results)



# revision 14
# speedup vs baseline: 1.5439x; 1.5439x over previous
"""Bass/Tile kernel for nn_CloudCrop (cylinder-query + gather + SharedMLP + max-pool).

Sharding: 8 cores = 4 batches x 2 query-halves. Each core handles 512 queries
against its batch's full 1024-point cloud.

Per-core pipeline (4 query-tiles of 128, software-pipelined A/B stages):
  stage A (mask + gather):
    1. PE: rotate all points into each query's frame (fp32r matmuls, K=3).
    2. ACT: scaled squares; DVE: g = max(y^2+z^2, x'^2); ACT sign (+count).
    3. DVE: score = desc * sign; 4x (max8 + match_replace) extracts the
       first 32 in-cylinder point indices; idx staged through DRAM into the
       16-partition-wrapped layout.
    4. SWDGE dma_gather x4 on queues 0-3 (SBUF->SBUF, transpose): Z rows
       (256ch bf16 + xyz) for 1024 samples each, channel-major.
    5. 16 small SBUF->SBUF DMAs write gathered xyz into the diagonal blocks
       of the per-tile resident block-diag rhs (const rows preloaded).
  stage B (MLP):
    6. PE: per-query A = R @ w1x^T/RAD (fp32r, K=3); y1 = relu(Zg + A.x + b1)
       via identity-inject + block-diag matmul.
    7. PE: layer 2 (bf16); DVE: max over 32 samples; ACT: relu + bias; DMA out.
Issue order A0 A1 B0 A2 B1 A3 B2 B3 overlaps tile t+1's mask/gather with
tile t's MLP.
"""
import sys

_RL = "/opt/trn_rl_repo"
if _RL not in sys.path:
    sys.path.insert(0, _RL)

from contextlib import ExitStack

import numpy as np
import ml_dtypes

import concourse.bass as bass
import concourse.bacc as bacc
import concourse.mybir as mybir
import concourse.tile as tile
from concourse import library_config

F32 = mybir.dt.float32
F32R = mybir.dt.float32r
BF16 = mybir.dt.bfloat16
F16 = mybir.dt.float16
I16 = mybir.dt.int16
U8 = mybir.dt.uint8
AL = mybir.AluOpType
AF = mybir.ActivationFunctionType
AX = mybir.AxisListType

RADIUS = 0.05
HMIN = -0.02
HMAX = 0.04
NS = 32
CIN = 512
CMID = 256
COUT = 256
EPS = 1e-5
B = 4
NQ = 1024
M = 1024
NT = 4          # query tiles per core
QPC = 512       # queries per core
G8 = 8          # groups per query tile
QG = 16         # queries per group
CX = (HMIN + HMAX) / 2.0
HH = (HMAX - HMIN) / 2.0
NCORES = 8
GSPLIT = 1      # gather sub-gathers (SWDGE queues)

bf16 = ml_dtypes.bfloat16


def build_nc(_stage=4, reps=1, gsplit=GSPLIT, qrot=1):
    nc = bacc.Bacc("TRN2", target_bir_lowering=False, num_devices=NCORES,
                   num_swdge_queues=max(gsplit, qrot))
    I = {}

    def din(name, shape, dt):
        I[name] = nc.dram_tensor(name, shape, dt, kind="ExternalInput").ap()

    din("xyzT", [3, M], F32R)
    din("xyzP", [128, 8, 3], F32)
    din("feat", [CIN, M], F32R)
    din("w1fT", [128, 4, CMID], F32R)
    din("w1xA", [3, CMID], F32R)
    din("RrowT", [3, NT, 3, 128], F32R)
    din("RcolT", [3, NT, G8, 6 * QG], F32R)
    din("Rq9", [128, NT, 9], F32)
    din("cQ9", [128, NT, 9], F32)
    din("rbdc", [96, NT, G8 * 512], BF16)
    din("w2T", [128, 2, 2, 128], BF16)
    din("b1", [128, 2], F32)
    din("b2", [128, 2], F32)
    din("desc16", [128, M], F16)
    din("siota", [128, NS], F32)
    din("qglob", [128, NT, NS], I16)
    din("ident", [128, 128], BF16)
    OUT = nc.dram_tensor("out", [COUT, QPC], F32, kind="ExternalOutput").ap()

    with ExitStack() as ctx:
        tc = ctx.enter_context(tile.TileContext(nc))
        res = ctx.enter_context(tc.tile_pool(name="res", bufs=1))
        wk = ctx.enter_context(tc.tile_pool(name="wk", bufs=2))
        wka = ctx.enter_context(tc.tile_pool(name="wka", bufs=1))
        wkg = ctx.enter_context(tc.tile_pool(name="wkg", bufs=2))
        drm = ctx.enter_context(tc.tile_pool(name="drm", bufs=2, space="DRAM"))
        p_rot = ctx.enter_context(tc.tile_pool(name="prot", bufs=1, space="PSUM"))
        p_y1 = ctx.enter_context(tc.tile_pool(name="py1", bufs=2, space="PSUM"))
        p_y2 = ctx.enter_context(tc.tile_pool(name="py2", bufs=2, space="PSUM"))
        p_sm = ctx.enter_context(tc.tile_pool(name="psml", bufs=2, space="PSUM"))

        nc.gpsimd.load_library(library_config.mlp)

        # ---------- residents ----------
        def rload(name, shape, dt, src=None):
            t = res.tile(shape, dt, tag="res_" + name)
            nc.sync.dma_start(out=t[:], in_=src if src is not None else I[name])
            return t

        xyzT_s = rload("xyzT", [3, M], F32R)
        w1fT_s = rload("w1fT", [128, 4, CMID], F32R)
        w1xA_s = rload("w1xA", [3, CMID], F32R)
        RrowT_s = rload("RrowT", [3, NT, 3, 128], F32R)
        RcolT_s = rload("RcolT", [3, NT, G8, 6 * QG], F32R)
        Rq9_s = rload("Rq9", [128, NT, 9], F32)
        cQ9_s = rload("cQ9", [128, NT, 9], F32)
        w2_s = rload("w2T", [128, 2, 2, 128], BF16)
        b1_s = rload("b1", [128, 2], F32)
        b2_s = rload("b2", [128, 2], F32)
        desc_s = rload("desc16", [128, M], F16)
        siota_s = rload("siota", [128, NS], F32)
        qglob_s = rload("qglob", [128, NT, NS], I16)
        ident_s = rload("ident", [128, 128], BF16)
        # per-tile block-diag rhs: const rows resident, xyz rows DMA'd per tile
        rbd_s = rload("rbdc", [96, NT, G8 * 512], BF16)

        feat_s = res.tile([128, 4, M], F32R, tag='res_feat')
        for kc in range(4):
            nc.sync.dma_start(out=feat_s[:, kc], in_=I["feat"][kc * 128:(kc + 1) * 128, :])

        # ---------- Z table (point-major, bf16): [m%128, m//128, 384] ----------
        repctx = tc.For_i(0, reps, 1) if reps > 1 else None
        if repctx is not None:
            repctx.__enter__()
        ZT = res.tile([128, 8, 384], BF16, tag='res_ZT')
        nc.scalar.memzero(ZT[:])
        for mt in range(8):
            pz = p_sm.tile([128, CMID], F32, tag="sm")
            for kc in range(4):
                nc.tensor.matmul(out=pz[:], lhsT=feat_s[:, kc, mt * 128:(mt + 1) * 128],
                                 rhs=w1fT_s[:, kc], start=(kc == 0), stop=(kc == 3))
            nc.scalar.copy(out=ZT[:, mt, 0:CMID], in_=pz[:])
        nc.gpsimd.dma_start(out=ZT[:, :, CMID:CMID + 3], in_=I["xyzP"])

        gz_t = [None] * NT

        def stage_a(t):
            # Rc[k] = sum_j R[k,j] * c[j]
            rc9 = wk.tile([128, 3, 3], F32, tag="rc9")
            nc.vector.tensor_tensor(
                out=rc9[:],
                in0=Rq9_s[:, t].rearrange("p (a b) -> p a b", a=3),
                in1=cQ9_s[:, t].rearrange("p (a b) -> p a b", a=3),
                op=AL.mult)
            rcb = wk.tile([128, 8], F32, tag="rcb")
            nc.vector.tensor_reduce(out=rcb[:, 0:3], in_=rc9[:], axis=AX.X, op=AL.add)
            nc.vector.tensor_scalar(out=rcb[:, 3:4], in0=rcb[:, 0:1],
                                    scalar1=-1.0 / HH, scalar2=-CX / HH,
                                    op0=AL.mult, op1=AL.add)
            nc.vector.tensor_scalar(out=rcb[:, 4:6], in0=rcb[:, 1:3],
                                    scalar1=-1.0 / RADIUS, scalar2=None, op0=AL.mult)

            # rotations + scaled squares
            sq = wka.tile([128, 3, M], F32, tag="sq")
            for r in range(3):
                pr = p_rot.tile([128, M], F32, tag="rot")
                for h in range(2):
                    nc.tensor.matmul(out=pr[:, h * 512:(h + 1) * 512],
                                     lhsT=RrowT_s[:, t, r],
                                     rhs=xyzT_s[:, h * 512:(h + 1) * 512],
                                     start=True, stop=True)
                scale = (1.0 / HH) if r == 0 else (1.0 / RADIUS)
                nc.scalar.activation(out=sq[:, r], in_=pr[:], func=AF.Square,
                                     bias=rcb[:, 3 + r:4 + r], scale=scale)

            # g = max(y2+z2, x2); sign; score; count
            tgg = wka.tile([128, 2, M], F32, tag="tg")
            nc.vector.tensor_tensor(out=tgg[:, 0], in0=sq[:, 1], in1=sq[:, 2], op=AL.add)
            nc.vector.tensor_tensor(out=tgg[:, 1], in0=tgg[:, 0], in1=sq[:, 0], op=AL.max)
            sgn = wk.tile([128, M], F16, tag="sgn")
            cnt_acc = wk.tile([128, 1], F32, tag="cnta")
            nc.scalar.activation(out=sgn[:], in_=tgg[:, 1], func=AF.Sign,
                                 bias=1.0, scale=-1.0, accum_out=cnt_acc[:])
            score = wk.tile([128, M], F16, tag="score")
            nc.vector.tensor_tensor(out=score[:], in0=desc_s[:], in1=sgn[:], op=AL.mult)

            # extract first-32 indices
            v32 = wk.tile([128, NS], F16, tag="v32")
            for r4 in range(4):
                nc.vector.max(out=v32[:, r4 * 8:(r4 + 1) * 8], in_=score[:])
                if r4 < 3:
                    nc.vector.match_replace(out=score[:],
                                            in_to_replace=v32[:, r4 * 8:(r4 + 1) * 8],
                                            in_values=score[:], imm_value=-3000.0)
            nc.vector.tensor_scalar(out=rcb[:, 6:7], in0=cnt_acc[:],
                                    scalar1=0.5, scalar2=512.0, op0=AL.mult, op1=AL.add)
            cond = wk.tile([128, NS], U8, tag="cond")
            nc.vector.tensor_scalar(out=cond[:], in0=siota_s[:], scalar1=rcb[:, 6:7],
                                    scalar2=None, op0=AL.is_lt)
            mvf = wk.tile([128, NS], F32, tag="mvf")
            nc.vector.tensor_scalar(out=mvf[:], in0=v32[:], scalar1=-1.0, scalar2=2048.0,
                                    op0=AL.mult, op1=AL.add)
            mvi = wk.tile([128, NS], I16, tag="mvi")
            nc.vector.tensor_scalar(out=mvi[:], in0=mvf[:], scalar1=1023.0, scalar2=None,
                                    op0=AL.min)
            idx16 = wk.tile([128, NS], I16, tag="idx16")
            nc.vector.tensor_copy(out=idx16[:], in_=qglob_s[:, t])
            nc.vector.copy_predicated(out=idx16[:], mask=cond[:], data=mvi[:])

            if _stage <= 1:
                dbg = wk.tile([128, NS], F32, tag="dbg")
                nc.vector.tensor_copy(out=dbg[:], in_=idx16[:])
                nc.sync.dma_start(out=OUT[0:128, t * NS:(t + 1) * NS], in_=dbg[:])
                return
            # stage roundtrip -> wrapped idx layout, replicated to 8 core-blocks
            stage = drm.tile([128, NS], I16, tag="stage")
            nc.sync.dma_start(out=stage[:], in_=idx16[:])
            wrap = stage[:].rearrange("q (h p) -> p (q h)", h=2, p=16)
            idxw = wk.tile([128, 256], I16, tag="idxw")
            for blk in range(8):
                nc.sync.dma_start(out=idxw[blk * 16:(blk + 1) * 16, :], in_=wrap)

            # gather: channel-major quarters [c%128, h, c//128, sample];
            # row 2 parts 0-2 = xyz; quarter h = queries 32h..32h+31
            gz = wkg.tile([128, gsplit, 3, NT * M // gsplit], BF16, tag="gz")
            gz_t[t] = gz
            npart = NT * M // gsplit
            for h in range(gsplit):
                nc.gpsimd.dma_gather(
                    out_ap=gz[:, h], in_ap=ZT[:],
                    idxs_ap=idxw[:, h * (npart // 16):(h + 1) * (npart // 16)],
                    num_idxs=npart, num_idxs_reg=npart, elem_size=384,
                    transpose=True, sbuf_tokens_per_rank=128,
                    sbuf_free_dim_per_rank=768, single_packet=False,
                    queue_num=(h if gsplit > 1 else t % qrot))

            if _stage <= 2:
                return
            # compact the per-quarter xyz rows, then scatter into the
            # block-diag diagonal (rows 6q'..6q'+2)
            gxc = wk.tile([3, NT * M], BF16, tag="gxc")
            nc.sync.dma_start(out=gxc[:], in_=gz[0:3, :, 2, :])
            rbdv = rbd_s[:, t].rearrange("p (g q s) -> p g q s", g=G8, q=QG, s=NS)
            gxv = gxc[:].rearrange("p (g q s) -> p g q s", g=G8, q=QG, s=NS)
            for qp in range(QG):
                eng = nc.sync if qp % 2 == 0 else nc.scalar
                eng.dma_start(out=rbdv[6 * qp:6 * qp + 3, :, qp],
                              in_=gxv[:, :, qp])

        def stage_b(t):
            if _stage <= 1:
                return
            if _stage in (2, 21):
                gz = gz_t[t]
                npart = NT * M // gsplit
                off = 0 if _stage == 2 else 1024
                h, c = off // npart, off % npart
                dbg2 = wk.tile([128, 128], F32, tag="dbg2")
                nc.vector.tensor_copy(out=dbg2[:], in_=gz[:, h, 0, c:c + 128])
                nc.sync.dma_start(out=OUT[0:128, t * 128:(t + 1) * 128], in_=dbg2[:])
                return
            gz = gz_t[t]
            # A matrices, [row=(6q'+jj), o]
            Ab = wk.tile([96, G8, CMID], BF16, tag="Ab")
            for g in range(G8):
                pa = p_sm.tile([128, CMID], F32, tag="sm")
                nc.tensor.matmul(out=pa[0:96, :], lhsT=RcolT_s[:, t, g], rhs=w1xA_s[:],
                                 start=True, stop=True)
                nc.scalar.copy(out=Ab[:, g], in_=pa[0:96, :])

            rbd3 = rbd_s[:, t].rearrange("p (g c) -> p g c", g=G8)
            # y1 = relu(Zg + A.x + b1)
            y1 = wka.tile([128, 2, NT * M], BF16, tag="y1")
            for oc in range(2):
                for g in range(G8):
                    py1 = p_y1.tile([128, 512], F32, tag="y1p")
                    npart = NT * M // gsplit
                    h, c = (g * 512) // npart, (g * 512) % npart
                    nc.tensor.matmul(out=py1[:], lhsT=ident_s[:],
                                     rhs=gz[:, h, oc, c:c + 512],
                                     start=True, stop=False)
                    nc.tensor.matmul(out=py1[:], lhsT=Ab[:, g, oc * 128:(oc + 1) * 128],
                                     rhs=rbd3[:, g], start=False, stop=True)
                    nc.scalar.activation(out=y1[:, oc, g * 512:(g + 1) * 512], in_=py1[:],
                                         func=AF.Relu, bias=b1_s[:, oc:oc + 1], scale=1.0)

            if _stage in (3, 31):
                off = 0 if _stage == 3 else 512
                dbg3 = wk.tile([128, 128], F32, tag="dbg3")
                nc.vector.tensor_copy(out=dbg3[:], in_=y1[:, 0, off:off + 128])
                nc.sync.dma_start(out=OUT[0:128, t * 128:(t + 1) * 128], in_=dbg3[:])
                return
            # layer 2 + max over 32 samples + relu+bias
            mx = wk.tile([128, 2, 128], F32, tag="mx")
            for oc2 in range(2):
                for g in range(G8):
                    py2 = p_y2.tile([128, 512], F32, tag="y2p")
                    for kc in range(2):
                        nc.tensor.matmul(
                            out=py2[:],
                            lhsT=w2_s[:, kc, oc2],
                            rhs=y1[:, kc, g * 512:(g + 1) * 512],
                            start=(kc == 0), stop=(kc == 1))
                    nc.vector.tensor_reduce(
                        out=mx[:, oc2, g * QG:(g + 1) * QG],
                        in_=py2[:].rearrange("p (q s) -> p q s", s=NS),
                        axis=AX.X, op=AL.max)
            outv = wk.tile([128, 2, 128], F32, tag="outv")
            for oc2 in range(2):
                nc.scalar.activation(out=outv[:, oc2], in_=mx[:, oc2], func=AF.Relu,
                                     bias=b2_s[:, oc2:oc2 + 1], scale=1.0)
                nc.sync.dma_start(out=OUT[oc2 * 128:(oc2 + 1) * 128, t * 128:(t + 1) * 128],
                                  in_=outv[:, oc2])

        # software pipeline: A0 A1 B0 A2 B1 A3 B2 B3
        stage_a(0)
        for t in range(1, NT):
            stage_a(t)
            stage_b(t - 1)
        stage_b(NT - 1)

        if repctx is not None:
            repctx.__exit__(None, None, None)
    return nc


def prep_inputs(inputs):
    xyz = np.asarray(inputs["seed_xyz"], dtype=np.float32)
    feat = np.asarray(inputs["seed_features"], dtype=np.float32)
    rot = np.asarray(inputs["vp_rot"], dtype=np.float32)

    def fold(g, b, m, v):
        s = (np.asarray(g, np.float32) /
             np.sqrt(np.asarray(v, np.float32) + np.float32(EPS))).astype(np.float32)
        return s, (np.asarray(b, np.float32) - np.asarray(m, np.float32) * s).astype(np.float32)

    s1, bb1 = fold(inputs["g1"], inputs["b1"], inputs["m1"], inputs["v1"])
    s2, bb2 = fold(inputs["g2"], inputs["b2"], inputs["m2"], inputs["v2"])
    w1s = (np.asarray(inputs["w1"], np.float32) * s1[:, None]).astype(np.float32)
    w1x, w1f = w1s[:, :3], w1s[:, 3:]
    w2s = (np.asarray(inputs["w2"], np.float32) * s2[:, None]).astype(np.float32)

    desc16 = np.broadcast_to(
        (2048.0 - np.arange(M, dtype=np.float32)).astype(np.float16), (128, M)).copy()
    siota = np.broadcast_to(np.arange(NS, dtype=np.float32), (128, NS)).copy()
    ident = np.eye(128, dtype=np.float32).astype(bf16)
    w2T = np.ascontiguousarray(
        w2s.T.reshape(2, 128, 2, 128).transpose(1, 0, 2, 3)).astype(bf16)
    w1fT = np.ascontiguousarray(w1f.T.reshape(4, 128, CMID).transpose(1, 0, 2))
    w1xA = np.ascontiguousarray(w1x.T / np.float32(RADIUS))
    b1h = np.ascontiguousarray(bb1.reshape(2, 128).T)
    b2h = np.ascontiguousarray(bb2.reshape(2, 128).T)

    ins = []
    for c in range(NCORES):
        b, half = c // 2, c % 2
        X = xyz[b]
        Rt = rot[b]
        qs = slice(half * QPC, (half + 1) * QPC)
        Rq = Rt[qs]          # (512, 3, 3)  R[j, k] (einsum 'bnsj,bnjk')
        cq = X[qs]           # (512, 3)
        d = {}
        d["xyzT"] = np.ascontiguousarray(X.T)
        d["xyzP"] = np.ascontiguousarray(X.reshape(8, 128, 3).transpose(1, 0, 2))
        d["feat"] = feat[b]
        d["w1fT"] = w1fT
        d["w1xA"] = w1xA
        # cylinder-query rotation: x_k = sum_j rot[k, j] rel_j
        Rr = Rq.reshape(NT, 128, 3, 3)   # [t, q, k(row), j(col)]
        d["RrowT"] = np.ascontiguousarray(Rr.transpose(3, 0, 2, 1))  # [j, t, k, q]
        # layer-1 rotation A[j, o] = sum_k rot[j, k] w1x[o, k] / RAD
        Rg = Rq.reshape(NT, G8, QG, 3, 3)  # [t, g, q', j, k]
        rcol = np.zeros((3, NT, G8, 6 * QG), np.float32)
        for jj in range(6):
            rcol[:, :, :, jj::6] = Rg[:, :, :, jj % 3, :].transpose(3, 0, 1, 2)
        d["RcolT"] = rcol
        d["Rq9"] = np.ascontiguousarray(Rq.reshape(NT, 128, 9).transpose(1, 0, 2))
        d["cQ9"] = np.ascontiguousarray(
            np.tile(cq.reshape(NT, 128, 3), (1, 1, 3)).transpose(1, 0, 2))
        # block-diag rhs const rows: rbdc[6q'+3+jj, t, g*512+q'*32+s] = -c_q[jj]
        rbdc = np.zeros((96, NT, G8, QG, NS), np.float32)
        cq4 = cq.reshape(NT, G8, QG, 3)          # [t, g, q', jj]
        for qp in range(QG):
            for jj in range(3):
                rbdc[6 * qp + 3 + jj, :, :, qp, :] = -cq4[:, :, qp, jj][:, :, None]
        d["rbdc"] = rbdc.reshape(96, NT, G8 * 512).astype(bf16)
        d["w2T"] = w2T
        d["b1"] = b1h
        d["b2"] = b2h
        d["desc16"] = desc16
        d["siota"] = siota
        qg = (half * QPC + np.arange(QPC, dtype=np.int16)).reshape(NT, 128)
        d["qglob"] = np.ascontiguousarray(
            np.broadcast_to(qg.T[:, :, None], (128, NT, NS))).astype(np.int16)
        d["ident"] = ident
        ins.append(d)
    return ins


def assemble(results):
    out = np.zeros((B, COUT, NQ), np.float32)
    for c in range(NCORES):
        b, half = c // 2, c % 2
        out[b, :, half * QPC:(half + 1) * QPC] = results[c]["out"]
    return out


_COMPILED = None


def _get_compiled():
    global _COMPILED
    if _COMPILED is None:
        nc = build_nc()
        nc.compile()
        _COMPILED = nc
    return _COMPILED


def kernel(**inputs):
    """Full-input entry point: shards across 8 NeuronCores, returns (B, 256, N) f32."""
    from concourse.bass_utils import run_bass_kernel_spmd
    nc = _get_compiled()
    ins = prep_inputs(inputs)
    res = run_bass_kernel_spmd(nc, ins, core_ids=list(range(NCORES)))
    return assemble(res.results)


# revision 15
# speedup vs baseline: 1.9423x; 1.2581x over previous
"""Bass/Tile kernel for nn_CloudCrop (cylinder-query + gather + SharedMLP + max-pool).

Sharding: 8 cores = 4 batches x 2 query-halves. Each core handles 512 queries
against its batch's full 1024-point cloud.

Per-core pipeline (4 query-tiles of 128, software-pipelined A/B stages):
  stage A (mask + gather):
    1. PE: rotate all points into each query's frame (fp32r matmuls, K=3).
    2. ACT: scaled squares; DVE: g = max(y^2+z^2, x'^2); ACT sign (+count).
    3. DVE: score = desc * sign; 4x (max8 + match_replace) extracts the
       first 32 in-cylinder point indices; idx staged through DRAM into the
       16-partition-wrapped layout.
    4. SWDGE dma_gather x4 on queues 0-3 (SBUF->SBUF, transpose): Z rows
       (256ch bf16 + xyz) for 1024 samples each, channel-major.
    5. 16 small SBUF->SBUF DMAs write gathered xyz into the diagonal blocks
       of the per-tile resident block-diag rhs (const rows preloaded).
  stage B (MLP):
    6. PE: per-query A = R @ w1x^T/RAD (fp32r, K=3); y1 = relu(Zg + A.x + b1)
       via identity-inject + block-diag matmul.
    7. PE: layer 2 (bf16); DVE: max over 32 samples; ACT: relu + bias; DMA out.
Issue order A0 A1 B0 A2 B1 A3 B2 B3 overlaps tile t+1's mask/gather with
tile t's MLP.
"""
import sys

_RL = "/opt/trn_rl_repo"
if _RL not in sys.path:
    sys.path.insert(0, _RL)

from contextlib import ExitStack

import numpy as np
import ml_dtypes

import concourse.bass as bass
import concourse.bacc as bacc
import concourse.mybir as mybir
import concourse.tile as tile
from concourse import library_config

F32 = mybir.dt.float32
F32R = mybir.dt.float32r
BF16 = mybir.dt.bfloat16
F16 = mybir.dt.float16
I16 = mybir.dt.int16
U8 = mybir.dt.uint8
AL = mybir.AluOpType
AF = mybir.ActivationFunctionType
AX = mybir.AxisListType

RADIUS = 0.05
HMIN = -0.02
HMAX = 0.04
NS = 32
CIN = 512
CMID = 256
COUT = 256
EPS = 1e-5
B = 4
NQ = 1024
M = 1024
NT = 4          # query tiles per core
QPC = 512       # queries per core
G8 = 8          # groups per query tile
QG = 16         # queries per group
CX = (HMIN + HMAX) / 2.0
HH = (HMAX - HMIN) / 2.0
NCORES = 8
GSPLIT = 1      # gather sub-gathers (SWDGE queues)

bf16 = ml_dtypes.bfloat16


def build_nc(_stage=4, reps=1, gsplit=GSPLIT, qrot=4):
    nc = bacc.Bacc("TRN2", target_bir_lowering=False, num_devices=NCORES,
                   num_swdge_queues=max(gsplit, qrot))
    I = {}

    def din(name, shape, dt):
        I[name] = nc.dram_tensor(name, shape, dt, kind="ExternalInput").ap()

    din("xyzT", [3, M], F32R)
    din("xyzP", [128, 8, 3], F32)
    din("feat", [CIN, M], F32R)
    din("w1fT", [128, 4, CMID], F32R)
    din("w1xA", [3, CMID], F32R)
    din("RrowT", [3, NT, 3, 128], F32R)
    din("RcolT", [3, NT, G8, 6 * QG], F32R)
    din("Rq9", [128, NT, 9], F32)
    din("cQ9", [128, NT, 9], F32)
    din("rbdc", [96, NT, G8 * 512], BF16)
    din("w2T", [128, 2, 2, 128], BF16)
    din("b1", [128, 2], F32)
    din("b2", [128, 2], F32)
    din("desc16", [128, M], F16)
    din("siota", [128, NS], F32)
    din("qglob", [128, NT, NS], I16)
    din("ident", [128, 128], BF16)
    OUT = nc.dram_tensor("out", [COUT, QPC], F32, kind="ExternalOutput").ap()

    with ExitStack() as ctx:
        tc = ctx.enter_context(tile.TileContext(nc))
        res = ctx.enter_context(tc.tile_pool(name="res", bufs=1))
        wk = ctx.enter_context(tc.tile_pool(name="wk", bufs=2))
        wka = ctx.enter_context(tc.tile_pool(name="wka", bufs=1))
        wkg = ctx.enter_context(tc.tile_pool(name="wkg", bufs=2))
        drm = ctx.enter_context(tc.tile_pool(name="drm", bufs=2, space="DRAM"))
        p_rot = ctx.enter_context(tc.tile_pool(name="prot", bufs=1, space="PSUM"))
        p_y1 = ctx.enter_context(tc.tile_pool(name="py1", bufs=2, space="PSUM"))
        p_y2 = ctx.enter_context(tc.tile_pool(name="py2", bufs=2, space="PSUM"))
        p_sm = ctx.enter_context(tc.tile_pool(name="psml", bufs=2, space="PSUM"))

        nc.gpsimd.load_library(library_config.mlp)

        # ---------- residents ----------
        def rload(name, shape, dt, src=None):
            t = res.tile(shape, dt, tag="res_" + name)
            nc.sync.dma_start(out=t[:], in_=src if src is not None else I[name])
            return t

        xyzT_s = rload("xyzT", [3, M], F32R)
        w1fT_s = rload("w1fT", [128, 4, CMID], F32R)
        w1xA_s = rload("w1xA", [3, CMID], F32R)
        RrowT_s = rload("RrowT", [3, NT, 3, 128], F32R)
        RcolT_s = rload("RcolT", [3, NT, G8, 6 * QG], F32R)
        Rq9_s = rload("Rq9", [128, NT, 9], F32)
        cQ9_s = rload("cQ9", [128, NT, 9], F32)
        w2_s = rload("w2T", [128, 2, 2, 128], BF16)
        b1_s = rload("b1", [128, 2], F32)
        b2_s = rload("b2", [128, 2], F32)
        desc_s = rload("desc16", [128, M], F16)
        siota_s = rload("siota", [128, NS], F32)
        qglob_s = rload("qglob", [128, NT, NS], I16)
        ident_s = rload("ident", [128, 128], BF16)
        # per-tile block-diag rhs: const rows resident, xyz rows DMA'd per tile
        rbd_s = rload("rbdc", [96, NT, G8 * 512], BF16)

        feat_s = res.tile([128, 4, M], F32R, tag='res_feat')
        for kc in range(4):
            nc.sync.dma_start(out=feat_s[:, kc], in_=I["feat"][kc * 128:(kc + 1) * 128, :])

        # ---------- Z table (point-major, bf16): [m%128, m//128, 384] ----------
        repctx = tc.For_i(0, reps, 1) if reps > 1 else None
        if repctx is not None:
            repctx.__enter__()
        ZT = res.tile([128, 8, 384], BF16, tag='res_ZT')
        nc.scalar.memzero(ZT[:])
        for mt in range(8):
            pz = p_sm.tile([128, CMID], F32, tag="sm")
            for kc in range(4):
                nc.tensor.matmul(out=pz[:], lhsT=feat_s[:, kc, mt * 128:(mt + 1) * 128],
                                 rhs=w1fT_s[:, kc], start=(kc == 0), stop=(kc == 3))
            nc.scalar.copy(out=ZT[:, mt, 0:CMID], in_=pz[:])
        nc.gpsimd.dma_start(out=ZT[:, :, CMID:CMID + 3], in_=I["xyzP"])

        gz_t = [None] * NT

        def stage_a(t):
            # Rc[k] = sum_j R[k,j] * c[j]
            rc9 = wk.tile([128, 3, 3], F32, tag="rc9")
            nc.vector.tensor_tensor(
                out=rc9[:],
                in0=Rq9_s[:, t].rearrange("p (a b) -> p a b", a=3),
                in1=cQ9_s[:, t].rearrange("p (a b) -> p a b", a=3),
                op=AL.mult)
            rcb = wk.tile([128, 8], F32, tag="rcb")
            nc.vector.tensor_reduce(out=rcb[:, 0:3], in_=rc9[:], axis=AX.X, op=AL.add)
            nc.vector.tensor_scalar(out=rcb[:, 3:4], in0=rcb[:, 0:1],
                                    scalar1=-1.0 / HH, scalar2=-CX / HH,
                                    op0=AL.mult, op1=AL.add)
            nc.vector.tensor_scalar(out=rcb[:, 4:6], in0=rcb[:, 1:3],
                                    scalar1=-1.0 / RADIUS, scalar2=None, op0=AL.mult)

            # rotations + scaled squares
            sq = wka.tile([128, 3, M], F32, tag="sq")
            for r in range(3):
                pr = p_rot.tile([128, M], F32, tag="rot")
                for h in range(2):
                    nc.tensor.matmul(out=pr[:, h * 512:(h + 1) * 512],
                                     lhsT=RrowT_s[:, t, r],
                                     rhs=xyzT_s[:, h * 512:(h + 1) * 512],
                                     start=True, stop=True)
                scale = (1.0 / HH) if r == 0 else (1.0 / RADIUS)
                nc.scalar.activation(out=sq[:, r], in_=pr[:], func=AF.Square,
                                     bias=rcb[:, 3 + r:4 + r], scale=scale)

            # g = max(y2+z2, x2); sign; score; count
            tgg = wka.tile([128, 2, M], F32, tag="tg")
            nc.vector.tensor_tensor(out=tgg[:, 0], in0=sq[:, 1], in1=sq[:, 2], op=AL.add)
            nc.vector.tensor_tensor(out=tgg[:, 1], in0=tgg[:, 0], in1=sq[:, 0], op=AL.max)
            sgn = wk.tile([128, M], F16, tag="sgn")
            cnt_acc = wk.tile([128, 1], F32, tag="cnta")
            nc.scalar.activation(out=sgn[:], in_=tgg[:, 1], func=AF.Sign,
                                 bias=1.0, scale=-1.0, accum_out=cnt_acc[:])
            score = wk.tile([128, M], F16, tag="score")
            nc.vector.tensor_tensor(out=score[:], in0=desc_s[:], in1=sgn[:], op=AL.mult)

            # extract first-32 indices
            v32 = wk.tile([128, NS], F16, tag="v32")
            for r4 in range(4):
                nc.vector.max(out=v32[:, r4 * 8:(r4 + 1) * 8], in_=score[:])
                if r4 < 3:
                    nc.vector.match_replace(out=score[:],
                                            in_to_replace=v32[:, r4 * 8:(r4 + 1) * 8],
                                            in_values=score[:], imm_value=-3000.0)
            nc.vector.tensor_scalar(out=rcb[:, 6:7], in0=cnt_acc[:],
                                    scalar1=0.5, scalar2=512.0, op0=AL.mult, op1=AL.add)
            cond = wk.tile([128, NS], U8, tag="cond")
            nc.vector.tensor_scalar(out=cond[:], in0=siota_s[:], scalar1=rcb[:, 6:7],
                                    scalar2=None, op0=AL.is_lt)
            mvf = wk.tile([128, NS], F32, tag="mvf")
            nc.vector.tensor_scalar(out=mvf[:], in0=v32[:], scalar1=-1.0, scalar2=2048.0,
                                    op0=AL.mult, op1=AL.add)
            mvi = wk.tile([128, NS], I16, tag="mvi")
            nc.vector.tensor_scalar(out=mvi[:], in0=mvf[:], scalar1=1023.0, scalar2=None,
                                    op0=AL.min)
            idx16 = wk.tile([128, NS], I16, tag="idx16")
            nc.vector.tensor_copy(out=idx16[:], in_=qglob_s[:, t])
            nc.vector.copy_predicated(out=idx16[:], mask=cond[:], data=mvi[:])

            if _stage <= 1:
                dbg = wk.tile([128, NS], F32, tag="dbg")
                nc.vector.tensor_copy(out=dbg[:], in_=idx16[:])
                nc.sync.dma_start(out=OUT[0:128, t * NS:(t + 1) * NS], in_=dbg[:])
                return
            # stage roundtrip -> wrapped idx layout, replicated to 8 core-blocks
            stage = drm.tile([128, NS], I16, tag="stage")
            nc.sync.dma_start(out=stage[:], in_=idx16[:])
            wrap = stage[:].rearrange("q (h p) -> p (q h)", h=2, p=16)
            idxw = wk.tile([128, 256], I16, tag="idxw")
            for blk in range(8):
                nc.sync.dma_start(out=idxw[blk * 16:(blk + 1) * 16, :], in_=wrap)

            # gather: channel-major quarters [c%128, h, c//128, sample];
            # row 2 parts 0-2 = xyz; quarter h = queries 32h..32h+31
            gz = wkg.tile([128, gsplit, 3, NT * M // gsplit], BF16, tag="gz")
            gz_t[t] = gz
            npart = NT * M // gsplit
            for h in range(gsplit):
                nc.gpsimd.dma_gather(
                    out_ap=gz[:, h], in_ap=ZT[:],
                    idxs_ap=idxw[:, h * (npart // 16):(h + 1) * (npart // 16)],
                    num_idxs=npart, num_idxs_reg=npart, elem_size=384,
                    transpose=True, sbuf_tokens_per_rank=128,
                    sbuf_free_dim_per_rank=768, single_packet=False,
                    queue_num=(h if gsplit > 1 else t % qrot))

            if _stage <= 2:
                return
            # compact the per-quarter xyz rows, then scatter into the
            # block-diag diagonal (rows 6q'..6q'+2)
            gxc = wk.tile([3, NT * M], BF16, tag="gxc")
            nc.sync.dma_start(out=gxc[:], in_=gz[0:3, :, 2, :])
            rbdv = rbd_s[:, t].rearrange("p (g q s) -> p g q s", g=G8, q=QG, s=NS)
            gxv = gxc[:].rearrange("p (g q s) -> p g q s", g=G8, q=QG, s=NS)
            for qp in range(QG):
                eng = nc.sync if qp % 2 == 0 else nc.scalar
                eng.dma_start(out=rbdv[6 * qp:6 * qp + 3, :, qp],
                              in_=gxv[:, :, qp])

        def stage_b(t):
            if _stage <= 1:
                return
            if _stage in (2, 21):
                gz = gz_t[t]
                npart = NT * M // gsplit
                off = 0 if _stage == 2 else 1024
                h, c = off // npart, off % npart
                dbg2 = wk.tile([128, 128], F32, tag="dbg2")
                nc.vector.tensor_copy(out=dbg2[:], in_=gz[:, h, 0, c:c + 128])
                nc.sync.dma_start(out=OUT[0:128, t * 128:(t + 1) * 128], in_=dbg2[:])
                return
            gz = gz_t[t]
            # A matrices, [row=(6q'+jj), o]
            Ab = wk.tile([96, G8, CMID], BF16, tag="Ab")
            for g in range(G8):
                pa = p_sm.tile([128, CMID], F32, tag="sm")
                nc.tensor.matmul(out=pa[0:96, :], lhsT=RcolT_s[:, t, g], rhs=w1xA_s[:],
                                 start=True, stop=True)
                nc.scalar.copy(out=Ab[:, g], in_=pa[0:96, :])

            rbd3 = rbd_s[:, t].rearrange("p (g c) -> p g c", g=G8)
            # y1 = relu(Zg + A.x + b1)
            y1 = wka.tile([128, 2, NT * M], BF16, tag="y1")
            for oc in range(2):
                for g in range(G8):
                    py1 = p_y1.tile([128, 512], F32, tag="y1p")
                    npart = NT * M // gsplit
                    h, c = (g * 512) // npart, (g * 512) % npart
                    nc.tensor.matmul(out=py1[:], lhsT=ident_s[:],
                                     rhs=gz[:, h, oc, c:c + 512],
                                     start=True, stop=False)
                    nc.tensor.matmul(out=py1[:], lhsT=Ab[:, g, oc * 128:(oc + 1) * 128],
                                     rhs=rbd3[:, g], start=False, stop=True)
                    nc.scalar.activation(out=y1[:, oc, g * 512:(g + 1) * 512], in_=py1[:],
                                         func=AF.Relu, bias=b1_s[:, oc:oc + 1], scale=1.0)

            if _stage in (3, 31):
                off = 0 if _stage == 3 else 512
                dbg3 = wk.tile([128, 128], F32, tag="dbg3")
                nc.vector.tensor_copy(out=dbg3[:], in_=y1[:, 0, off:off + 128])
                nc.sync.dma_start(out=OUT[0:128, t * 128:(t + 1) * 128], in_=dbg3[:])
                return
            # layer 2 + max over 32 samples + relu+bias
            mx = wk.tile([128, 2, 128], F32, tag="mx")
            for oc2 in range(2):
                for g in range(G8):
                    py2 = p_y2.tile([128, 512], F32, tag="y2p")
                    for kc in range(2):
                        nc.tensor.matmul(
                            out=py2[:],
                            lhsT=w2_s[:, kc, oc2],
                            rhs=y1[:, kc, g * 512:(g + 1) * 512],
                            start=(kc == 0), stop=(kc == 1))
                    nc.vector.tensor_reduce(
                        out=mx[:, oc2, g * QG:(g + 1) * QG],
                        in_=py2[:].rearrange("p (q s) -> p q s", s=NS),
                        axis=AX.X, op=AL.max)
            outv = wk.tile([128, 2, 128], F32, tag="outv")
            for oc2 in range(2):
                nc.scalar.activation(out=outv[:, oc2], in_=mx[:, oc2], func=AF.Relu,
                                     bias=b2_s[:, oc2:oc2 + 1], scale=1.0)
                nc.sync.dma_start(out=OUT[oc2 * 128:(oc2 + 1) * 128, t * 128:(t + 1) * 128],
                                  in_=outv[:, oc2])

        # software pipeline: A0 A1 B0 A2 B1 A3 B2 B3
        stage_a(0)
        for t in range(1, NT):
            stage_a(t)
            stage_b(t - 1)
        stage_b(NT - 1)

        if repctx is not None:
            repctx.__exit__(None, None, None)
    return nc


def prep_inputs(inputs):
    xyz = np.asarray(inputs["seed_xyz"], dtype=np.float32)
    feat = np.asarray(inputs["seed_features"], dtype=np.float32)
    rot = np.asarray(inputs["vp_rot"], dtype=np.float32)

    def fold(g, b, m, v):
        s = (np.asarray(g, np.float32) /
             np.sqrt(np.asarray(v, np.float32) + np.float32(EPS))).astype(np.float32)
        return s, (np.asarray(b, np.float32) - np.asarray(m, np.float32) * s).astype(np.float32)

    s1, bb1 = fold(inputs["g1"], inputs["b1"], inputs["m1"], inputs["v1"])
    s2, bb2 = fold(inputs["g2"], inputs["b2"], inputs["m2"], inputs["v2"])
    w1s = (np.asarray(inputs["w1"], np.float32) * s1[:, None]).astype(np.float32)
    w1x, w1f = w1s[:, :3], w1s[:, 3:]
    w2s = (np.asarray(inputs["w2"], np.float32) * s2[:, None]).astype(np.float32)

    desc16 = np.broadcast_to(
        (2048.0 - np.arange(M, dtype=np.float32)).astype(np.float16), (128, M)).copy()
    siota = np.broadcast_to(np.arange(NS, dtype=np.float32), (128, NS)).copy()
    ident = np.eye(128, dtype=np.float32).astype(bf16)
    w2T = np.ascontiguousarray(
        w2s.T.reshape(2, 128, 2, 128).transpose(1, 0, 2, 3)).astype(bf16)
    w1fT = np.ascontiguousarray(w1f.T.reshape(4, 128, CMID).transpose(1, 0, 2))
    w1xA = np.ascontiguousarray(w1x.T / np.float32(RADIUS))
    b1h = np.ascontiguousarray(bb1.reshape(2, 128).T)
    b2h = np.ascontiguousarray(bb2.reshape(2, 128).T)

    ins = []
    for c in range(NCORES):
        b, half = c // 2, c % 2
        X = xyz[b]
        Rt = rot[b]
        qs = slice(half * QPC, (half + 1) * QPC)
        Rq = Rt[qs]          # (512, 3, 3)  R[j, k] (einsum 'bnsj,bnjk')
        cq = X[qs]           # (512, 3)
        d = {}
        d["xyzT"] = np.ascontiguousarray(X.T)
        d["xyzP"] = np.ascontiguousarray(X.reshape(8, 128, 3).transpose(1, 0, 2))
        d["feat"] = feat[b]
        d["w1fT"] = w1fT
        d["w1xA"] = w1xA
        # cylinder-query rotation: x_k = sum_j rot[k, j] rel_j
        Rr = Rq.reshape(NT, 128, 3, 3)   # [t, q, k(row), j(col)]
        d["RrowT"] = np.ascontiguousarray(Rr.transpose(3, 0, 2, 1))  # [j, t, k, q]
        # layer-1 rotation A[j, o] = sum_k rot[j, k] w1x[o, k] / RAD
        Rg = Rq.reshape(NT, G8, QG, 3, 3)  # [t, g, q', j, k]
        rcol = np.zeros((3, NT, G8, 6 * QG), np.float32)
        for jj in range(6):
            rcol[:, :, :, jj::6] = Rg[:, :, :, jj % 3, :].transpose(3, 0, 1, 2)
        d["RcolT"] = rcol
        d["Rq9"] = np.ascontiguousarray(Rq.reshape(NT, 128, 9).transpose(1, 0, 2))
        d["cQ9"] = np.ascontiguousarray(
            np.tile(cq.reshape(NT, 128, 3), (1, 1, 3)).transpose(1, 0, 2))
        # block-diag rhs const rows: rbdc[6q'+3+jj, t, g*512+q'*32+s] = -c_q[jj]
        rbdc = np.zeros((96, NT, G8, QG, NS), np.float32)
        cq4 = cq.reshape(NT, G8, QG, 3)          # [t, g, q', jj]
        for qp in range(QG):
            for jj in range(3):
                rbdc[6 * qp + 3 + jj, :, :, qp, :] = -cq4[:, :, qp, jj][:, :, None]
        d["rbdc"] = rbdc.reshape(96, NT, G8 * 512).astype(bf16)
        d["w2T"] = w2T
        d["b1"] = b1h
        d["b2"] = b2h
        d["desc16"] = desc16
        d["siota"] = siota
        qg = (half * QPC + np.arange(QPC, dtype=np.int16)).reshape(NT, 128)
        d["qglob"] = np.ascontiguousarray(
            np.broadcast_to(qg.T[:, :, None], (128, NT, NS))).astype(np.int16)
        d["ident"] = ident
        ins.append(d)
    return ins


def assemble(results):
    out = np.zeros((B, COUT, NQ), np.float32)
    for c in range(NCORES):
        b, half = c // 2, c % 2
        out[b, :, half * QPC:(half + 1) * QPC] = results[c]["out"]
    return out


_COMPILED = None


def _get_compiled():
    global _COMPILED
    if _COMPILED is None:
        nc = build_nc()
        nc.compile()
        _COMPILED = nc
    return _COMPILED


def kernel(**inputs):
    """Full-input entry point: shards across 8 NeuronCores, returns (B, 256, N) f32."""
    from concourse.bass_utils import run_bass_kernel_spmd
    nc = _get_compiled()
    ins = prep_inputs(inputs)
    res = run_bass_kernel_spmd(nc, ins, core_ids=list(range(NCORES)))
    return assemble(res.results)
